# revision 4
# baseline (speedup 1.0000x reference)
"""Trainium2 Bass kernel for the AttentionConvBlock problem (fp8 conv edition).

Reference computation (per batch b of 8):
    q = relu(conv3x3(x, Wq) + bq); k = relu(conv3x3(x, Wk) + bk)
    v = conv3x3(x, Wv) + bv
    S = q @ k (per-channel [128,128] spatial matmul)
    P = softmax over flattened 16384 entries per (b, c)
    y = P @ v + x

Sharding: data-parallel over batch, one batch per NeuronCore (8 cores).

Per-core plan (vs the bf16 baseline, conv matmuls move to fp8 DoubleRow):
  - Host pre-quantizes x to fp8e4 (TRN E4M3, |x|max ~5 << 240) and uploads
    BOTH padded layouts (natural [h,w] and transposed [w,h]) with zero
    borders baked in: no on-device casts, no transpose pass, 4x less
    startup DMA than the fp32 staging path.
  - Weights are scaled by 2^13 (uniform(-1/48, 1/48) -> +-170, centered in
    e4m3 range) and packed per shift as [ic=128, icc=2, oc=128] fp8 tiles;
    the 2^-13 descale rides the PSUM-evacuation activation's scale.
  - Conv as 9-shift DoubleRow matmul: each shift contracts BOTH 128-channel
    input chunks at once (lhsT [128,2,128], rhs [128,2,L] 3-D APs, the PE
    virtualizes to 128x256). 9 matmuls/chunk instead of 18 at ~2 MACs/cell.
    Position chunks are FLAT runs of the padded image (3 rows x 130 = 390
    free dim <= 512 PSUM limit); the 2-column seam junk is simply never
    evacuated (strided ScalarE read of the valid 128 columns).
  - Chunks run in pairs sharing each shift's weight load (halves LDWEIGHTS
    pressure, which is 2x in DoubleRow); psum pool of 4 banks keeps the
    pair pipeline full while ScalarE drains evacuations.
  - q/k round-trip HBM in fp8 (attention S matmul runs fp8 at bf16 rate),
    v in bf16 (avoids mixed-dtype y matmul); P=exp(S) stays bf16.
  - Attention structure unchanged from the baseline: per 4-channel wave,
    4 S-matmuls into one PSUM bank, batched exp, DVE sums, all-ones-matmul
    global-sum broadcast, 4 y-matmuls, ScalarE 1/Z scale, DVE residual add.
    S and Y waves share one 3-bank PSUM tag ring. Block-0 attention is
    woven into block-1's conv stream; block-1's attention is the tail.
"""
import os
import sys

sys.path.insert(0, "/opt/trn_rl_repo")
os.environ.setdefault("MYCRO_LOCAL_CACHE", "1")

import numpy as np
import ml_dtypes

B, C, H, W = 8, 256, 128, 128
HW = H * W
N_CORES = 8
PAD = 130            # padded row/col length
PADSZ = PAD * PAD    # 16900 valid bytes per icc copy
PADSTRIDE = 16912    # icc stride, padded to %16 for the DoubleRow AP rule
WSCALE = 2.0 ** 13

# 43 position chunks per conv block: 42 x 3 rows + 1 x 2 rows
CHUNKS = [(r0, 3) for r0 in range(0, 126, 3)] + [(126, 2)]
GROUPS = [CHUNKS[i : i + 2] for i in range(0, len(CHUNKS), 2)]  # 21 pairs + single

_PROG = None


def _build_program():
    import concourse.bass as bass
    import concourse.tile as tile
    from concourse import bacc, mybir

    dt = mybir.dt
    AF = mybir.ActivationFunctionType
    DR = mybir.MatmulPerfMode.DoubleRow

    nc = bacc.Bacc("TRN2", target_bir_lowering=False, debug=False)

    x_d = nc.dram_tensor("x", [C, H, W], dt.float32, kind="ExternalInput").ap()
    x8n_d = nc.dram_tensor("x8n", [2, 128, PADSTRIDE], dt.float8e4, kind="ExternalInput").ap()
    x8t_d = nc.dram_tensor("x8t", [2, 128, PADSTRIDE], dt.float8e4, kind="ExternalInput").ap()
    w_d = nc.dram_tensor("wpack", [54, 128, 256], dt.float8e4, kind="ExternalInput").ap()
    b_d = nc.dram_tensor("bpack", [128, 6], dt.float32, kind="ExternalInput").ap()
    y_d = nc.dram_tensor("y", [C, H, W], dt.float32, kind="ExternalOutput").ap()

    with tile.TileContext(nc) as tc:
        from contextlib import ExitStack

        with ExitStack() as ctx:
            const = ctx.enter_context(tc.tile_pool(name="const", bufs=1))
            xpad_p = ctx.enter_context(tc.tile_pool(name="xpad", bufs=1))
            evac = ctx.enter_context(tc.tile_pool(name="evac", bufs=4))
            qload = ctx.enter_context(tc.tile_pool(name="qload", bufs=4))
            kload = ctx.enter_context(tc.tile_pool(name="kload", bufs=4))
            vload = ctx.enter_context(tc.tile_pool(name="vload", bufs=4))
            att = ctx.enter_context(tc.tile_pool(name="att", bufs=2))
            stat = ctx.enter_context(tc.tile_pool(name="stat", bufs=3))
            resid = ctx.enter_context(tc.tile_pool(name="resid", bufs=3))
            outp = ctx.enter_context(tc.tile_pool(name="outp", bufs=3))
            psum_c = ctx.enter_context(tc.tile_pool(name="psc", bufs=4, space="PSUM"))
            psum_a = ctx.enter_context(tc.tile_pool(name="psa", bufs=3, space="PSUM"))
            psum_z = ctx.enter_context(tc.tile_pool(name="psz", bufs=1, space="PSUM"))
            dram = ctx.enter_context(tc.tile_pool(name="dram", bufs=1, space="DRAM"))

            # ---- constants ----
            w_sb = const.tile([128, 54, 256], dt.float8e4)
            b_sb = const.tile([128, 6], dt.float32)
            nc.sync.dma_start(out=b_sb[:], in_=b_d)
            ones_bf = const.tile([128, 128], dt.bfloat16)
            nc.vector.memset(ones_bf[:], 1.0)

            # ---- padded fp8 x, natural and transposed (host pre-padded) ----
            xa = xpad_p.tile([128, 2, PADSTRIDE], dt.float8e4, tag="xa")
            xt = xpad_p.tile([128, 2, PADSTRIDE], dt.float8e4, tag="xt")
            NST = PADSTRIDE // 16  # 1057-byte stage slices
            for s in range(16):
                for icc in range(2):
                    q = nc.sync if (s + icc) % 2 == 0 else nc.gpsimd
                    q.dma_start(
                        out=xa[:, icc, s * NST : (s + 1) * NST],
                        in_=x8n_d[icc, :, s * NST : (s + 1) * NST],
                    )
                if s == 0:
                    nc.sync.dma_start(
                        out=w_sb[:, 0:9, :], in_=w_d[0:9].rearrange("t p f -> p t f")
                    )
                if s == 1:
                    nc.sync.dma_start(
                        out=w_sb[:, 9:27, :], in_=w_d[9:27].rearrange("t p f -> p t f")
                    )
            for s in range(16):
                for icc in range(2):
                    q = nc.sync if (s + icc) % 2 == 0 else nc.gpsimd
                    q.dma_start(
                        out=xt[:, icc, s * NST : (s + 1) * NST],
                        in_=x8t_d[icc, :, s * NST : (s + 1) * NST],
                    )
            nc.sync.dma_start(
                out=w_sb[:, 27:54, :], in_=w_d[27:54].rearrange("t p f -> p t f")
            )

            # ---- HBM round-trip buffers: position-major [j, c, i] ----
            qt_dram = dram.tile([128, C, 128], dt.float8e4, tag="qt")
            k_dram = dram.tile([128, C, 128], dt.float8e4, tag="kd")
            v_dram = dram.tile([128, C, 128], dt.bfloat16, tag="vd")
            cv_dram = [k_dram, v_dram, qt_dram]  # cvslot order: k, v, q

            def conv_group(occ, cvslot, chunks):
                # one PSUM bank per chunk; shifts share each weight load
                src = xt if cvslot == 2 else xa
                ps = [
                    psum_c.tile([128, 3, PAD], dt.float32, tag="psc", name=f"psc{ci}")
                    for ci in range(len(chunks))
                ]
                for kk in range(9):
                    dy, dx = kk // 3, kk % 3
                    w3 = w_sb[:, occ * 27 + cvslot * 9 + kk, :].rearrange(
                        "p (two o) -> p two o", two=2
                    )
                    for ci, (r0, nr) in enumerate(chunks):
                        s0 = (r0 + dy) * PAD + dx
                        nc.tensor.matmul(
                            ps[ci][:, 0:nr, :],
                            lhsT=w3,
                            rhs=src[:, :, s0 : s0 + nr * PAD],
                            start=(kk == 0),
                            stop=(kk == 8),
                            perf_mode=DR,
                        )
                for ci, (r0, nr) in enumerate(chunks):
                    if cvslot == 1:
                        ev = evac.tile([128, 3, 128], dt.bfloat16, tag="evv")
                    else:
                        ev = evac.tile([128, 3, 128], dt.float8e4, tag="evqk")
                    nc.scalar.activation(
                        out=ev[:, 0:nr, :],
                        in_=ps[ci][:, 0:nr, 0:128],
                        func=AF.Identity if cvslot == 1 else AF.Relu,
                        bias=b_sb[:, occ * 3 + cvslot : occ * 3 + cvslot + 1],
                        scale=1.0 / WSCALE,
                    )
                    nc.sync.dma_start(
                        out=cv_dram[cvslot][
                            r0 : r0 + nr, occ * 128 : (occ + 1) * 128, :
                        ].rearrange("j c i -> c j i"),
                        in_=ev[:, 0:nr, :],
                    )

            # ---- attention (baseline structure; q/k fp8, v bf16) ----
            def att_load(occ, g0):
                c0 = occ * 128 + g0
                qt8 = qload.tile([128, 8, 128], dt.float8e4, tag="qt8")
                nc.sync.dma_start(out=qt8[:], in_=qt_dram[:, c0 : c0 + 8, :])
                k8 = kload.tile([128, 8, 128], dt.float8e4, tag="k8")
                nc.sync.dma_start(out=k8[:], in_=k_dram[:, c0 : c0 + 8, :])
                v8 = vload.tile([128, 8, 128], dt.bfloat16, tag="v8")
                nc.gpsimd.dma_start(out=v8[:], in_=v_dram[:, c0 : c0 + 8, :])
                return qt8, k8, v8

            def att_wave(occ, g0, tiles, w):
                qt8, k8, v8 = tiles
                o = 4 * w
                c0 = occ * 128 + g0 + o
                xr4 = resid.tile([128, 4, 128], dt.float32, tag="xr4")
                nc.gpsimd.dma_start(
                    out=xr4[:], in_=x_d[c0 : c0 + 4].rearrange("c h w -> h c w")
                )
                ps_s = psum_a.tile([128, 4, 128], dt.float32, tag="psa")
                for j in range(4):
                    nc.tensor.matmul(
                        ps_s[:, j, :],
                        lhsT=k8[:, o + j, :],
                        rhs=qt8[:, o + j, :],
                        start=True,
                        stop=True,
                    )
                p4 = att.tile([128, 4, 128], dt.bfloat16, tag="p4")
                nc.scalar.activation(out=p4[:], in_=ps_s[:], func=AF.Exp)
                cs4 = stat.tile([128, 4], dt.float32, tag="cs4")
                nc.vector.reduce_sum(cs4[:], p4[:], axis=mybir.AxisListType.X)
                cs4b = stat.tile([128, 4], dt.bfloat16, tag="cs4b")
                nc.vector.tensor_copy(cs4b[:], cs4[:])
                ps_z = psum_z.tile([128, 4], dt.float32, tag="psz")
                nc.tensor.matmul(
                    ps_z[:], lhsT=ones_bf[:], rhs=cs4b[:], start=True, stop=True
                )
                rec4 = stat.tile([128, 4], dt.float32, tag="rec4")
                nc.vector.reciprocal(rec4[:], ps_z[:])
                ps_y = psum_a.tile([128, 4, 128], dt.float32, tag="psa")
                for j in range(4):
                    nc.tensor.matmul(
                        ps_y[:, j, :],
                        lhsT=p4[:, j, :],
                        rhs=v8[:, o + j, :],
                        start=True,
                        stop=True,
                    )
                out4 = outp.tile([128, 4, 128], dt.float32, tag="out4")
                for j in range(4):
                    nc.scalar.activation(
                        out=out4[:, j, :],
                        in_=ps_y[:, j, :],
                        func=AF.Copy,
                        scale=rec4[:, j : j + 1],
                    )
                nc.vector.tensor_add(out4[:], out4[:], xr4[:])
                nc.gpsimd.dma_start(
                    out=y_d[c0 : c0 + 4].rearrange("c h w -> h c w"), in_=out4[:]
                )

            def att_steps(occ):
                for g0 in range(0, 128, 8):
                    tiles = []

                    def load(g0=g0, tiles=tiles):
                        tiles.append(att_load(occ, g0))

                    def wave0(g0=g0, tiles=tiles):
                        att_wave(occ, g0, tiles[0], 0)

                    def wave1(g0=g0, tiles=tiles):
                        att_wave(occ, g0, tiles[0], 1)

                    yield load
                    yield wave0
                    yield wave1

            # Phase A: block-0 convs (k, v, q order: xt arrives during k/v)
            for cvslot in range(3):
                for grp in GROUPS:
                    conv_group(0, cvslot, grp)
            # Phase B: block-1 convs with block-0 attention woven in
            steps = att_steps(0)
            for cvslot in range(3):
                for grp in GROUPS:
                    conv_group(1, cvslot, grp)
                    step = next(steps, None)
                    if step is not None:
                        step()
            for step in steps:
                step()
            # Phase C: block-1 attention tail
            for step in att_steps(1):
                step()

    nc.compile()
    return nc


def _get_program():
    global _PROG
    if _PROG is None:
        _PROG = _build_program()
    return _PROG


def _pack_weights(Wq, Wk, Wv):
    # w_d[t, ic, icc*128 + oc], t = occ*27 + cvslot*9 + kk (cvslot: k,v,q).
    # The q conv runs on the TRANSPOSED image with the same (dy,dx) shift
    # arithmetic, so its taps must be packed transposed.
    out = np.zeros((54, 128, 256), np.float32)
    for cvslot, Wcv in ((0, Wk), (1, Wv), (2, np.asarray(Wq).transpose(0, 1, 3, 2))):
        a = np.asarray(Wcv, np.float32) * WSCALE  # [ocg, icg, dy, dx]
        a = a.transpose(2, 3, 1, 0).reshape(9, 2, 128, 2, 128)  # kk,icc,ic,occ,oc
        a = a.transpose(3, 0, 2, 1, 4).reshape(2, 9, 128, 256)  # occ,kk,ic,(icc oc)
        for occ in range(2):
            base = occ * 27 + cvslot * 9
            out[base : base + 9] = a[occ]
    return np.clip(out, -240, 240).astype(ml_dtypes.float8_e4m3)


def _pack_x8(xb):
    # xb [256, 128, 128] fp32 -> padded fp8 natural + transposed layouts
    x8 = np.asarray(xb, np.float32).astype(ml_dtypes.float8_e4m3)
    x8 = x8.reshape(2, 128, H, W)
    nat = np.zeros((2, 128, PAD, PAD), ml_dtypes.float8_e4m3)
    nat[:, :, 1 : 1 + H, 1 : 1 + W] = x8
    tra = np.zeros((2, 128, PAD, PAD), ml_dtypes.float8_e4m3)
    tra[:, :, 1 : 1 + W, 1 : 1 + H] = x8.transpose(0, 1, 3, 2)
    full_n = np.zeros((2, 128, PADSTRIDE), ml_dtypes.float8_e4m3)
    full_n[:, :, :PADSZ] = nat.reshape(2, 128, PADSZ)
    full_t = np.zeros((2, 128, PADSTRIDE), ml_dtypes.float8_e4m3)
    full_t[:, :, :PADSZ] = tra.reshape(2, 128, PADSZ)
    return np.ascontiguousarray(full_n), np.ascontiguousarray(full_t)


def _run(inputs, trace=False, trace_kwargs=None):
    from concourse.bass_utils import run_bass_kernel_spmd

    nc = _get_program()
    x = np.ascontiguousarray(np.asarray(inputs["x"], np.float32))
    wpack = _pack_weights(inputs["Wq"], inputs["Wk"], inputs["Wv"])
    bq = np.asarray(inputs["bq"], np.float32)
    bk = np.asarray(inputs["bk"], np.float32)
    bv = np.asarray(inputs["bv"], np.float32)
    # col = occ*3 + cvslot, cvslot order (k, v, q)
    bpack = np.stack(
        [bk[:128], bv[:128], bq[:128], bk[128:], bv[128:], bq[128:]], axis=1
    )
    bpack = np.ascontiguousarray(bpack, dtype=np.float32)  # [128, 6]

    in_maps = []
    for b in range(N_CORES):
        x8n, x8t = _pack_x8(x[b])
        in_maps.append(
            {"x": x[b], "x8n": x8n, "x8t": x8t, "wpack": wpack, "bpack": bpack}
        )
    last_err = None
    for attempt in range(3):
        try:
            res = run_bass_kernel_spmd(
                nc,
                in_maps,
                core_ids=list(range(N_CORES)),
                trace=trace,
                **(trace_kwargs or {}),
            )
            break
        except Exception as e:  # transient device/runtime hiccups
            last_err = e
            if attempt == 2:
                raise
            import time

            time.sleep(5.0)
    out = np.stack([res.results[b]["y"] for b in range(N_CORES)], axis=0)
    return out, res


def kernel(**inputs) -> np.ndarray:
    out, _ = _run(inputs, trace=False)
    return out


def kernel_traced(inputs):
    try:
        import axon_shim

        axon_shim.install()
    except Exception:
        pass
    out, res = _run(inputs, trace=True)
    return out, res


# revision 11
# speedup vs baseline: 1.0054x; 1.0054x over previous
"""Trainium2 Bass kernel for the AttentionConvBlock problem (fp8 conv edition).

Reference computation (per batch b of 8):
    q = relu(conv3x3(x, Wq) + bq); k = relu(conv3x3(x, Wk) + bk)
    v = conv3x3(x, Wv) + bv
    S = q @ k (per-channel [128,128] spatial matmul)
    P = softmax over flattened 16384 entries per (b, c)
    y = P @ v + x

Sharding: data-parallel over batch, one batch per NeuronCore (8 cores).

Per-core plan (vs the bf16 baseline, conv matmuls move to fp8 DoubleRow):
  - Host pre-quantizes x to fp8e4 (TRN E4M3, |x|max ~5 << 240) and uploads
    BOTH padded layouts (natural [h,w] and transposed [w,h]) with zero
    borders baked in: no on-device casts, no transpose pass, 4x less
    startup DMA than the fp32 staging path.
  - Weights are scaled by 2^13 (uniform(-1/48, 1/48) -> +-170, centered in
    e4m3 range) and packed per shift as [ic=128, icc=2, oc=128] fp8 tiles;
    the 2^-13 descale rides the PSUM-evacuation activation's scale.
  - Conv as 9-shift DoubleRow matmul: each shift contracts BOTH 128-channel
    input chunks at once (lhsT [128,2,128], rhs [128,2,L] 3-D APs, the PE
    virtualizes to 128x256). 9 matmuls/chunk instead of 18 at ~2 MACs/cell.
    Position chunks are FLAT runs of the padded image (3 rows x 130 = 390
    free dim <= 512 PSUM limit); the 2-column seam junk is simply never
    evacuated (strided ScalarE read of the valid 128 columns).
  - Chunks run in pairs sharing each shift's weight load (halves LDWEIGHTS
    pressure, which is 2x in DoubleRow); psum pool of 4 banks keeps the
    pair pipeline full while ScalarE drains evacuations.
  - q/k round-trip HBM in fp8 (attention S matmul runs fp8 at bf16 rate),
    v in bf16 (avoids mixed-dtype y matmul); P=exp(S) stays bf16.
  - Attention structure unchanged from the baseline: per 4-channel wave,
    4 S-matmuls into one PSUM bank, batched exp, DVE sums, all-ones-matmul
    global-sum broadcast, 4 y-matmuls, ScalarE 1/Z scale, DVE residual add.
    S and Y waves share one 3-bank PSUM tag ring. Block-0 attention is
    woven into block-1's conv stream; block-1's attention is the tail.
"""
import os
import sys

sys.path.insert(0, "/opt/trn_rl_repo")
os.environ.setdefault("MYCRO_LOCAL_CACHE", "1")

import numpy as np
import ml_dtypes

B, C, H, W = 8, 256, 128, 128
HW = H * W
N_CORES = 8
PAD = 130            # padded row/col length
PADSZ = PAD * PAD    # 16900 valid bytes per icc copy
PADSTRIDE = 16912    # icc stride, padded to %16 for the DoubleRow AP rule
WSCALE = 2.0 ** 13

# 43 position chunks per conv block: 42 x 3 rows + 1 x 2 rows
CHUNKS = [(r0, 3) for r0 in range(0, 126, 3)] + [(126, 2)]
GROUPS = [CHUNKS[i : i + 2] for i in range(0, len(CHUNKS), 2)]  # 21 pairs + single

_PROG = None


def _build_program():
    import concourse.bass as bass
    import concourse.tile as tile
    from concourse import bacc, mybir

    dt = mybir.dt
    AF = mybir.ActivationFunctionType
    DR = mybir.MatmulPerfMode.DoubleRow

    nc = bacc.Bacc("TRN2", target_bir_lowering=False, debug=False)

    xbf_d = nc.dram_tensor("xbf", [C, H, W], dt.bfloat16, kind="ExternalInput").ap()
    x8n_d = nc.dram_tensor("x8n", [2, 128, PADSTRIDE], dt.float8e4, kind="ExternalInput").ap()
    x8t_d = nc.dram_tensor("x8t", [2, 128, PADSTRIDE], dt.float8e4, kind="ExternalInput").ap()
    w_d = nc.dram_tensor("wpack", [54, 128, 256], dt.float8e4, kind="ExternalInput").ap()
    b_d = nc.dram_tensor("bpack", [128, 6], dt.float32, kind="ExternalInput").ap()
    y_d = nc.dram_tensor("y", [C, H, W], dt.bfloat16, kind="ExternalOutput").ap()

    with tile.TileContext(nc) as tc:
        from contextlib import ExitStack

        with ExitStack() as ctx:
            const = ctx.enter_context(tc.tile_pool(name="const", bufs=1))
            xpad_p = ctx.enter_context(tc.tile_pool(name="xpad", bufs=1))
            evac = ctx.enter_context(tc.tile_pool(name="evac", bufs=4))
            qload = ctx.enter_context(tc.tile_pool(name="qload", bufs=4))
            kload = ctx.enter_context(tc.tile_pool(name="kload", bufs=4))
            vload = ctx.enter_context(tc.tile_pool(name="vload", bufs=4))
            att = ctx.enter_context(tc.tile_pool(name="att", bufs=2))
            stat = ctx.enter_context(tc.tile_pool(name="stat", bufs=3))
            resid = ctx.enter_context(tc.tile_pool(name="resid", bufs=3))
            outp = ctx.enter_context(tc.tile_pool(name="outp", bufs=3))
            psum_c = ctx.enter_context(tc.tile_pool(name="psc", bufs=4, space="PSUM"))
            psum_a = ctx.enter_context(tc.tile_pool(name="psa", bufs=3, space="PSUM"))
            psum_z = ctx.enter_context(tc.tile_pool(name="psz", bufs=1, space="PSUM"))
            dram = ctx.enter_context(tc.tile_pool(name="dram", bufs=1, space="DRAM"))

            # ---- constants ----
            w_sb = const.tile([128, 54, 256], dt.float8e4)
            b_sb = const.tile([128, 6], dt.float32)
            nc.scalar.dma_start(out=b_sb[:], in_=b_d)
            ones_bf = const.tile([128, 128], dt.bfloat16)
            nc.vector.memset(ones_bf[:], 1.0)

            # first shift's weights ahead of everything on the sync queue
            nc.sync.dma_start(
                out=w_sb[:, 0:1, :], in_=w_d[0:1].rearrange("t p f -> p t f")
            )

            # ---- padded fp8 x, natural and transposed (host pre-padded) ----
            # round-robin over all five engine DMA queues: the k conv burns
            # x at ~65 GB/s and two queues can't stay ahead of it
            xa = xpad_p.tile([128, 2, PADSTRIDE], dt.float8e4, tag="xa")
            xt = xpad_p.tile([128, 2, PADSTRIDE], dt.float8e4, tag="xt")
            queues = [nc.sync, nc.gpsimd, nc.scalar]
            NST = PADSTRIDE // 16  # 1057-byte stage slices
            qi = 0
            for s in range(16):
                for icc in range(2):
                    queues[qi % 3].dma_start(
                        out=xa[:, icc, s * NST : (s + 1) * NST],
                        in_=x8n_d[icc, :, s * NST : (s + 1) * NST],
                    )
                    qi += 1
                if s == 0:
                    nc.sync.dma_start(
                        out=w_sb[:, 1:9, :], in_=w_d[1:9].rearrange("t p f -> p t f")
                    )
                if s == 1:
                    nc.sync.dma_start(
                        out=w_sb[:, 9:27, :], in_=w_d[9:27].rearrange("t p f -> p t f")
                    )
            for s in range(16):
                for icc in range(2):
                    queues[qi % 3].dma_start(
                        out=xt[:, icc, s * NST : (s + 1) * NST],
                        in_=x8t_d[icc, :, s * NST : (s + 1) * NST],
                    )
                    qi += 1
            nc.sync.dma_start(
                out=w_sb[:, 27:54, :], in_=w_d[27:54].rearrange("t p f -> p t f")
            )

            # ---- HBM round-trip buffers: position-major [j, c, i] ----
            qt_dram = dram.tile([128, C, 128], dt.float8e4, tag="qt")
            k_dram = dram.tile([128, C, 128], dt.float8e4, tag="kd")
            v_dram = dram.tile([128, C, 128], dt.bfloat16, tag="vd")
            cv_dram = [k_dram, v_dram, qt_dram]  # cvslot order: k, v, q

            def conv_group(occ, cvslot, chunks):
                # one PSUM bank per chunk; shifts share each weight load
                src = xt if cvslot == 2 else xa
                ps = [
                    psum_c.tile([128, 3, PAD], dt.float32, tag="psc", name=f"psc{ci}")
                    for ci in range(len(chunks))
                ]
                for kk in range(9):
                    dy, dx = kk // 3, kk % 3
                    w3 = w_sb[:, occ * 27 + cvslot * 9 + kk, :].rearrange(
                        "p (two o) -> p two o", two=2
                    )
                    for ci, (r0, nr) in enumerate(chunks):
                        s0 = (r0 + dy) * PAD + dx
                        nc.tensor.matmul(
                            ps[ci][:, 0:nr, :],
                            lhsT=w3,
                            rhs=src[:, :, s0 : s0 + nr * PAD],
                            start=(kk == 0),
                            stop=(kk == 8),
                            perf_mode=DR,
                        )
                for ci, (r0, nr) in enumerate(chunks):
                    if cvslot == 1:
                        ev = evac.tile([128, 3, 128], dt.bfloat16, tag="evv")
                    else:
                        ev = evac.tile([128, 3, 128], dt.float8e4, tag="evqk")
                    nc.scalar.activation(
                        out=ev[:, 0:nr, :],
                        in_=ps[ci][:, 0:nr, 0:128],
                        func=AF.Identity if cvslot == 1 else AF.Relu,
                        bias=b_sb[:, occ * 3 + cvslot : occ * 3 + cvslot + 1],
                        scale=1.0 / WSCALE,
                    )
                    nc.sync.dma_start(
                        out=cv_dram[cvslot][
                            r0 : r0 + nr, occ * 128 : (occ + 1) * 128, :
                        ].rearrange("j c i -> c j i"),
                        in_=ev[:, 0:nr, :],
                    )

            # ---- attention (baseline structure; q/k fp8, v bf16) ----
            def att_load(occ, g0):
                c0 = occ * 128 + g0
                qt8 = qload.tile([128, 8, 128], dt.float8e4, tag="qt8")
                nc.sync.dma_start(out=qt8[:], in_=qt_dram[:, c0 : c0 + 8, :])
                k8 = kload.tile([128, 8, 128], dt.float8e4, tag="k8")
                nc.sync.dma_start(out=k8[:], in_=k_dram[:, c0 : c0 + 8, :])
                v8 = vload.tile([128, 8, 128], dt.bfloat16, tag="v8")
                nc.gpsimd.dma_start(out=v8[:], in_=v_dram[:, c0 : c0 + 8, :])
                xr8 = resid.tile([128, 8, 128], dt.bfloat16, tag="xr8")
                nc.scalar.dma_start(
                    out=xr8[:], in_=xbf_d[c0 : c0 + 8].rearrange("c h w -> h c w")
                )
                return qt8, k8, v8, xr8

            def att_wave(occ, g0, tiles, w):
                qt8, k8, v8, xr8 = tiles
                o = 4 * w
                c0 = occ * 128 + g0 + o
                ps_s = psum_a.tile([128, 4, 128], dt.float32, tag="psa")
                for j in range(4):
                    nc.tensor.matmul(
                        ps_s[:, j, :],
                        lhsT=k8[:, o + j, :],
                        rhs=qt8[:, o + j, :],
                        start=True,
                        stop=True,
                    )
                p4 = att.tile([128, 4, 128], dt.bfloat16, tag="p4")
                nc.scalar.activation(out=p4[:], in_=ps_s[:], func=AF.Exp)
                cs4 = stat.tile([128, 4], dt.float32, tag="cs4")
                nc.vector.reduce_sum(cs4[:], p4[:], axis=mybir.AxisListType.X)
                cs4b = stat.tile([128, 4], dt.bfloat16, tag="cs4b")
                nc.vector.tensor_copy(cs4b[:], cs4[:])
                ps_z = psum_z.tile([128, 4], dt.float32, tag="psz")
                nc.tensor.matmul(
                    ps_z[:], lhsT=ones_bf[:], rhs=cs4b[:], start=True, stop=True
                )
                rec4 = stat.tile([128, 4], dt.float32, tag="rec4")
                nc.vector.reciprocal(rec4[:], ps_z[:])
                ps_y = psum_a.tile([128, 4, 128], dt.float32, tag="psa")
                for j in range(4):
                    nc.tensor.matmul(
                        ps_y[:, j, :],
                        lhsT=p4[:, j, :],
                        rhs=v8[:, o + j, :],
                        start=True,
                        stop=True,
                    )
                out4 = outp.tile([128, 4, 128], dt.bfloat16, tag="out4")
                for j in range(4):
                    nc.scalar.activation(
                        out=out4[:, j, :],
                        in_=ps_y[:, j, :],
                        func=AF.Copy,
                        scale=rec4[:, j : j + 1],
                    )
                nc.vector.tensor_add(out4[:], out4[:], xr8[:, o : o + 4, :])
                nc.gpsimd.dma_start(
                    out=y_d[c0 : c0 + 4].rearrange("c h w -> h c w"), in_=out4[:]
                )

            def att_steps(occ):
                for g0 in range(0, 128, 8):
                    tiles = []

                    def load(g0=g0, tiles=tiles):
                        tiles.append(att_load(occ, g0))

                    def wave0(g0=g0, tiles=tiles):
                        att_wave(occ, g0, tiles[0], 0)

                    def wave1(g0=g0, tiles=tiles):
                        att_wave(occ, g0, tiles[0], 1)

                    yield load
                    yield wave0
                    yield wave1

            # Phase A: block-0 convs (k, v, q order: xt arrives during k/v)
            for cvslot in range(3):
                for grp in GROUPS:
                    conv_group(0, cvslot, grp)
            # Phase B: block-1 convs with block-0 attention woven in
            steps = att_steps(0)
            for cvslot in range(3):
                for grp in GROUPS:
                    conv_group(1, cvslot, grp)
                    step = next(steps, None)
                    if step is not None:
                        step()
            for step in steps:
                step()
            # Phase C: block-1 attention tail
            for step in att_steps(1):
                step()

    nc.compile()
    return nc


def _get_program():
    global _PROG
    if _PROG is None:
        _PROG = _build_program()
    return _PROG


def _pack_weights(Wq, Wk, Wv):
    # w_d[t, ic, icc*128 + oc], t = occ*27 + cvslot*9 + kk (cvslot: k,v,q).
    # The q conv runs on the TRANSPOSED image with the same (dy,dx) shift
    # arithmetic, so its taps must be packed transposed.
    out = np.zeros((54, 128, 256), np.float32)
    for cvslot, Wcv in ((0, Wk), (1, Wv), (2, np.asarray(Wq).transpose(0, 1, 3, 2))):
        a = np.asarray(Wcv, np.float32) * WSCALE  # [ocg, icg, dy, dx]
        a = a.transpose(2, 3, 1, 0).reshape(9, 2, 128, 2, 128)  # kk,icc,ic,occ,oc
        a = a.transpose(3, 0, 2, 1, 4).reshape(2, 9, 128, 256)  # occ,kk,ic,(icc oc)
        for occ in range(2):
            base = occ * 27 + cvslot * 9
            out[base : base + 9] = a[occ]
    return np.clip(out, -240, 240).astype(ml_dtypes.float8_e4m3)


def _pack_x8(xb):
    # xb [256, 128, 128] fp32 -> padded fp8 natural + transposed layouts
    x8 = np.asarray(xb, np.float32).astype(ml_dtypes.float8_e4m3)
    x8 = x8.reshape(2, 128, H, W)
    nat = np.zeros((2, 128, PAD, PAD), ml_dtypes.float8_e4m3)
    nat[:, :, 1 : 1 + H, 1 : 1 + W] = x8
    tra = np.zeros((2, 128, PAD, PAD), ml_dtypes.float8_e4m3)
    tra[:, :, 1 : 1 + W, 1 : 1 + H] = x8.transpose(0, 1, 3, 2)
    full_n = np.zeros((2, 128, PADSTRIDE), ml_dtypes.float8_e4m3)
    full_n[:, :, :PADSZ] = nat.reshape(2, 128, PADSZ)
    full_t = np.zeros((2, 128, PADSTRIDE), ml_dtypes.float8_e4m3)
    full_t[:, :, :PADSZ] = tra.reshape(2, 128, PADSZ)
    return np.ascontiguousarray(full_n), np.ascontiguousarray(full_t)


def _run(inputs, trace=False, trace_kwargs=None):
    from concourse.bass_utils import run_bass_kernel_spmd

    nc = _get_program()
    x = np.ascontiguousarray(np.asarray(inputs["x"], np.float32))
    wpack = _pack_weights(inputs["Wq"], inputs["Wk"], inputs["Wv"])
    bq = np.asarray(inputs["bq"], np.float32)
    bk = np.asarray(inputs["bk"], np.float32)
    bv = np.asarray(inputs["bv"], np.float32)
    # col = occ*3 + cvslot, cvslot order (k, v, q)
    bpack = np.stack(
        [bk[:128], bv[:128], bq[:128], bk[128:], bv[128:], bq[128:]], axis=1
    )
    bpack = np.ascontiguousarray(bpack, dtype=np.float32)  # [128, 6]

    in_maps = []
    for b in range(N_CORES):
        x8n, x8t = _pack_x8(x[b])
        xbf = np.ascontiguousarray(x[b].astype(ml_dtypes.bfloat16))
        in_maps.append(
            {"xbf": xbf, "x8n": x8n, "x8t": x8t, "wpack": wpack, "bpack": bpack}
        )
    last_err = None
    for attempt in range(3):
        try:
            res = run_bass_kernel_spmd(
                nc,
                in_maps,
                core_ids=list(range(N_CORES)),
                trace=trace,
                **(trace_kwargs or {}),
            )
            break
        except Exception as e:  # transient device/runtime hiccups
            last_err = e
            if attempt == 2:
                raise
            import time

            time.sleep(5.0)
    out = np.stack(
        [np.asarray(res.results[b]["y"], np.float32) for b in range(N_CORES)], axis=0
    )
    return out, res


def kernel(**inputs) -> np.ndarray:
    out, _ = _run(inputs, trace=False)
    return out


def kernel_traced(inputs):
    try:
        import axon_shim

        axon_shim.install()
    except Exception:
        pass
    out, res = _run(inputs, trace=True)
    return out, res


# revision 15
# speedup vs baseline: 1.0246x; 1.0191x over previous
"""Trainium2 Bass kernel for the AttentionConvBlock problem (fp8 conv edition).

Reference computation (per batch b of 8):
    q = relu(conv3x3(x, Wq) + bq); k = relu(conv3x3(x, Wk) + bk)
    v = conv3x3(x, Wv) + bv
    S = q @ k (per-channel [128,128] spatial matmul)
    P = softmax over flattened 16384 entries per (b, c)
    y = P @ v + x

Sharding: data-parallel over batch, one batch per NeuronCore (8 cores).

Per-core plan (vs the bf16 baseline, conv matmuls move to fp8 DoubleRow):
  - Host pre-quantizes x to fp8e4 (TRN E4M3, |x|max ~5 << 240) and uploads
    BOTH padded layouts (natural [h,w] and transposed [w,h]) with zero
    borders baked in: no on-device casts, no transpose pass, 4x less
    startup DMA than the fp32 staging path.
  - Weights are scaled by 2^13 (uniform(-1/48, 1/48) -> +-170, centered in
    e4m3 range) and packed per shift as [ic=128, icc=2, oc=128] fp8 tiles;
    the 2^-13 descale rides the PSUM-evacuation activation's scale.
  - Conv as 9-shift DoubleRow matmul: each shift contracts BOTH 128-channel
    input chunks at once (lhsT [128,2,128], rhs [128,2,L] 3-D APs, the PE
    virtualizes to 128x256). 9 matmuls/chunk instead of 18 at ~2 MACs/cell.
    Position chunks are FLAT runs of the padded image (3 rows x 130 = 390
    free dim <= 512 PSUM limit); the 2-column seam junk is simply never
    evacuated (strided ScalarE read of the valid 128 columns).
  - Chunks run in pairs sharing each shift's weight load (halves LDWEIGHTS
    pressure, which is 2x in DoubleRow); psum pool of 4 banks keeps the
    pair pipeline full while ScalarE drains evacuations.
  - q/k round-trip HBM in fp8 (attention S matmul runs fp8 at bf16 rate),
    v in bf16 (avoids mixed-dtype y matmul); P=exp(S) stays bf16.
  - Attention structure unchanged from the baseline: per 4-channel wave,
    4 S-matmuls into one PSUM bank, batched exp, DVE sums, all-ones-matmul
    global-sum broadcast, 4 y-matmuls, ScalarE 1/Z scale, DVE residual add.
    S and Y waves share one 3-bank PSUM tag ring. Block-0 attention is
    woven into block-1's conv stream; block-1's attention is the tail.
"""
import os
import sys

sys.path.insert(0, "/opt/trn_rl_repo")
os.environ.setdefault("MYCRO_LOCAL_CACHE", "1")

import numpy as np
import ml_dtypes

B, C, H, W = 8, 256, 128, 128
HW = H * W
N_CORES = 8
PAD = 130            # padded row/col length
PADSZ = PAD * PAD    # 16900 valid bytes per icc copy
PADSTRIDE = 16912    # icc stride, padded to %16 for the DoubleRow AP rule
WSCALE = 2.0 ** 13

# 43 position chunks per conv block: 42 x 3 rows + 1 x 2 rows
CHUNKS = [(r0, 3) for r0 in range(0, 126, 3)] + [(126, 2)]
GROUPS = [CHUNKS[i : i + 2] for i in range(0, len(CHUNKS), 2)]  # 21 pairs + single

_PROG = None


def _build_program():
    import concourse.bass as bass
    import concourse.tile as tile
    from concourse import bacc, mybir

    dt = mybir.dt
    AF = mybir.ActivationFunctionType
    DR = mybir.MatmulPerfMode.DoubleRow

    nc = bacc.Bacc("TRN2", target_bir_lowering=False, debug=False)

    # [H, C, W] so per-partition(h) runs are 8ch x 128 contiguous elements
    xbf_d = nc.dram_tensor("xbf", [H, C, W], dt.bfloat16, kind="ExternalInput").ap()
    x8n_d = nc.dram_tensor("x8n", [2, 128, PADSTRIDE], dt.float8e4, kind="ExternalInput").ap()
    x8t_d = nc.dram_tensor("x8t", [2, 128, PADSTRIDE], dt.float8e4, kind="ExternalInput").ap()
    w_d = nc.dram_tensor("wpack", [54, 128, 256], dt.float8e4, kind="ExternalInput").ap()
    b_d = nc.dram_tensor("bpack", [128, 6], dt.float32, kind="ExternalInput").ap()
    y_d = nc.dram_tensor("y", [H, C, W], dt.bfloat16, kind="ExternalOutput").ap()

    with tile.TileContext(nc) as tc:
        from contextlib import ExitStack

        with ExitStack() as ctx:
            const = ctx.enter_context(tc.tile_pool(name="const", bufs=1))
            xpad_p = ctx.enter_context(tc.tile_pool(name="xpad", bufs=1))
            evac = ctx.enter_context(tc.tile_pool(name="evac", bufs=4))
            qload = ctx.enter_context(tc.tile_pool(name="qload", bufs=6))
            kload = ctx.enter_context(tc.tile_pool(name="kload", bufs=6))
            vload = ctx.enter_context(tc.tile_pool(name="vload", bufs=6))
            att = ctx.enter_context(tc.tile_pool(name="att", bufs=2))
            stat = ctx.enter_context(tc.tile_pool(name="stat", bufs=3))
            resid = ctx.enter_context(tc.tile_pool(name="resid", bufs=6))
            outp = ctx.enter_context(tc.tile_pool(name="outp", bufs=3))
            psum_c = ctx.enter_context(tc.tile_pool(name="psc", bufs=4, space="PSUM"))
            psum_a = ctx.enter_context(tc.tile_pool(name="psa", bufs=3, space="PSUM"))
            psum_z = ctx.enter_context(tc.tile_pool(name="psz", bufs=1, space="PSUM"))
            dram = ctx.enter_context(tc.tile_pool(name="dram", bufs=1, space="DRAM"))

            # ---- constants ----
            w_sb = const.tile([128, 54, 256], dt.float8e4)
            b_sb = const.tile([128, 6], dt.float32)
            nc.scalar.dma_start(out=b_sb[:], in_=b_d)
            ones_bf = const.tile([128, 128], dt.bfloat16)
            nc.vector.memset(ones_bf[:], 1.0)

            # first shift's weights ahead of everything on the sync queue
            nc.sync.dma_start(
                out=w_sb[:, 0:1, :], in_=w_d[0:1].rearrange("t p f -> p t f")
            )

            # ---- padded fp8 x, natural and transposed (host pre-padded) ----
            # progressive slice sizes: small first slices so the conv can
            # start immediately, big later ones to keep descriptor count low
            # (descriptor gen/issue rate, not bytes, paced the old staging).
            # icc0 rides sync, icc1 rides gpsimd, in parallel.
            xa = xpad_p.tile([128, 2, PADSTRIDE], dt.float8e4, tag="xa")
            xt = xpad_p.tile([128, 2, PADSTRIDE], dt.float8e4, tag="xt")
            NST = PADSTRIDE // 16
            SLICES = [
                (0, NST),
                (NST, NST),
                (2 * NST, 2 * NST),
                (4 * NST, 4 * NST),
                (8 * NST, 8 * NST),
            ]
            for si, (o0, ln) in enumerate(SLICES):
                for icc, q in ((0, nc.sync), (1, nc.gpsimd)):
                    q.dma_start(
                        out=xa[:, icc, o0 : o0 + ln],
                        in_=x8n_d[icc, :, o0 : o0 + ln],
                    )
                if si == 0:
                    nc.sync.dma_start(
                        out=w_sb[:, 1:9, :], in_=w_d[1:9].rearrange("t p f -> p t f")
                    )
                    nc.gpsimd.dma_start(
                        out=w_sb[:, 9:27, :], in_=w_d[9:27].rearrange("t p f -> p t f")
                    )
            for o0, ln in SLICES:
                for icc, q in ((0, nc.scalar), (1, nc.gpsimd)):
                    q.dma_start(
                        out=xt[:, icc, o0 : o0 + ln],
                        in_=x8t_d[icc, :, o0 : o0 + ln],
                    )
            nc.scalar.dma_start(
                out=w_sb[:, 27:54, :], in_=w_d[27:54].rearrange("t p f -> p t f")
            )

            # ---- HBM round-trip buffers: position-major [j, c, i] ----
            qt_dram = dram.tile([128, C, 128], dt.float8e4, tag="qt")
            k_dram = dram.tile([128, C, 128], dt.float8e4, tag="kd")
            v_dram = dram.tile([128, C, 128], dt.bfloat16, tag="vd")
            cv_dram = [k_dram, v_dram, qt_dram]  # cvslot order: k, v, q

            def conv_group(occ, cvslot, chunks):
                # one PSUM bank per chunk; shifts share each weight load
                src = xt if cvslot == 2 else xa
                ps = [
                    psum_c.tile([128, 3, PAD], dt.float32, tag="psc", name=f"psc{ci}")
                    for ci in range(len(chunks))
                ]
                for kk in range(9):
                    dy, dx = kk // 3, kk % 3
                    w3 = w_sb[:, occ * 27 + cvslot * 9 + kk, :].rearrange(
                        "p (two o) -> p two o", two=2
                    )
                    for ci, (r0, nr) in enumerate(chunks):
                        s0 = (r0 + dy) * PAD + dx
                        nc.tensor.matmul(
                            ps[ci][:, 0:nr, :],
                            lhsT=w3,
                            rhs=src[:, :, s0 : s0 + nr * PAD],
                            start=(kk == 0),
                            stop=(kk == 8),
                            perf_mode=DR,
                        )
                for ci, (r0, nr) in enumerate(chunks):
                    if cvslot == 1:
                        ev = evac.tile([128, 3, 128], dt.bfloat16, tag="evv")
                    else:
                        ev = evac.tile([128, 3, 128], dt.float8e4, tag="evqk")
                    nc.scalar.activation(
                        out=ev[:, 0:nr, :],
                        in_=ps[ci][:, 0:nr, 0:128],
                        func=AF.Identity if cvslot == 1 else AF.Relu,
                        bias=b_sb[:, occ * 3 + cvslot : occ * 3 + cvslot + 1],
                        scale=1.0 / WSCALE,
                    )
                    nc.sync.dma_start(
                        out=cv_dram[cvslot][
                            r0 : r0 + nr, occ * 128 : (occ + 1) * 128, :
                        ].rearrange("j c i -> c j i"),
                        in_=ev[:, 0:nr, :],
                    )

            # ---- attention (baseline structure; q/k fp8, v bf16) ----
            def att_load(occ, g0):
                c0 = occ * 128 + g0
                qt8 = qload.tile([128, 8, 128], dt.float8e4, tag="qt8")
                nc.sync.dma_start(out=qt8[:], in_=qt_dram[:, c0 : c0 + 8, :])
                k8 = kload.tile([128, 8, 128], dt.float8e4, tag="k8")
                nc.sync.dma_start(out=k8[:], in_=k_dram[:, c0 : c0 + 8, :])
                v8 = vload.tile([128, 8, 128], dt.bfloat16, tag="v8")
                nc.gpsimd.dma_start(out=v8[:], in_=v_dram[:, c0 : c0 + 8, :])
                xr8 = resid.tile([128, 8, 128], dt.bfloat16, tag="xr8")
                nc.scalar.dma_start(out=xr8[:], in_=xbf_d[:, c0 : c0 + 8, :])
                return qt8, k8, v8, xr8

            def att_wave(occ, g0, tiles, w):
                qt8, k8, v8, xr8 = tiles
                o = 4 * w
                c0 = occ * 128 + g0 + o
                ps_s = psum_a.tile([128, 4, 128], dt.float32, tag="psa")
                for j in range(4):
                    nc.tensor.matmul(
                        ps_s[:, j, :],
                        lhsT=k8[:, o + j, :],
                        rhs=qt8[:, o + j, :],
                        start=True,
                        stop=True,
                    )
                p4 = att.tile([128, 4, 128], dt.bfloat16, tag="p4")
                nc.scalar.activation(out=p4[:], in_=ps_s[:], func=AF.Exp)
                cs4 = stat.tile([128, 4], dt.float32, tag="cs4")
                nc.vector.reduce_sum(cs4[:], p4[:], axis=mybir.AxisListType.X)
                cs4b = stat.tile([128, 4], dt.bfloat16, tag="cs4b")
                nc.vector.tensor_copy(cs4b[:], cs4[:])
                ps_z = psum_z.tile([128, 4], dt.float32, tag="psz")
                nc.tensor.matmul(
                    ps_z[:], lhsT=ones_bf[:], rhs=cs4b[:], start=True, stop=True
                )
                rec4 = stat.tile([128, 4], dt.float32, tag="rec4")
                nc.vector.reciprocal(rec4[:], ps_z[:])
                ps_y = psum_a.tile([128, 4, 128], dt.float32, tag="psa")
                for j in range(4):
                    nc.tensor.matmul(
                        ps_y[:, j, :],
                        lhsT=p4[:, j, :],
                        rhs=v8[:, o + j, :],
                        start=True,
                        stop=True,
                    )
                out4 = outp.tile([128, 4, 128], dt.bfloat16, tag="out4")
                for j in range(4):
                    nc.scalar.activation(
                        out=out4[:, j, :],
                        in_=ps_y[:, j, :],
                        func=AF.Copy,
                        scale=rec4[:, j : j + 1],
                    )
                nc.vector.tensor_add(out4[:], out4[:], xr8[:, o : o + 4, :])
                nc.gpsimd.dma_start(out=y_d[:, c0 : c0 + 4, :], in_=out4[:])

            def att_steps(occ):
                for g0 in range(0, 128, 8):
                    tiles = []

                    def load(g0=g0, tiles=tiles):
                        tiles.append(att_load(occ, g0))

                    def wave0(g0=g0, tiles=tiles):
                        att_wave(occ, g0, tiles[0], 0)

                    def wave1(g0=g0, tiles=tiles):
                        att_wave(occ, g0, tiles[0], 1)

                    yield load
                    yield wave0
                    yield wave1

            # Phase A: block-0 convs (k, v, q order: xt arrives during k/v)
            for cvslot in range(3):
                for grp in GROUPS:
                    conv_group(0, cvslot, grp)
            # Phase B: block-1 convs with block-0 attention woven in
            steps = att_steps(0)
            for cvslot in range(3):
                for grp in GROUPS:
                    conv_group(1, cvslot, grp)
                    step = next(steps, None)
                    if step is not None:
                        step()
            for step in steps:
                step()
            # Phase C: block-1 attention tail
            for step in att_steps(1):
                step()

    nc.compile()
    return nc


def _get_program():
    global _PROG
    if _PROG is None:
        _PROG = _build_program()
    return _PROG


def _pack_weights(Wq, Wk, Wv):
    # w_d[t, ic, icc*128 + oc], t = occ*27 + cvslot*9 + kk (cvslot: k,v,q).
    # The q conv runs on the TRANSPOSED image with the same (dy,dx) shift
    # arithmetic, so its taps must be packed transposed.
    out = np.zeros((54, 128, 256), np.float32)
    for cvslot, Wcv in ((0, Wk), (1, Wv), (2, np.asarray(Wq).transpose(0, 1, 3, 2))):
        a = np.asarray(Wcv, np.float32) * WSCALE  # [ocg, icg, dy, dx]
        a = a.transpose(2, 3, 1, 0).reshape(9, 2, 128, 2, 128)  # kk,icc,ic,occ,oc
        a = a.transpose(3, 0, 2, 1, 4).reshape(2, 9, 128, 256)  # occ,kk,ic,(icc oc)
        for occ in range(2):
            base = occ * 27 + cvslot * 9
            out[base : base + 9] = a[occ]
    return np.clip(out, -240, 240).astype(ml_dtypes.float8_e4m3)


def _pack_x8(xb):
    # xb [256, 128, 128] fp32 -> padded fp8 natural + transposed layouts
    x8 = np.asarray(xb, np.float32).astype(ml_dtypes.float8_e4m3)
    x8 = x8.reshape(2, 128, H, W)
    nat = np.zeros((2, 128, PAD, PAD), ml_dtypes.float8_e4m3)
    nat[:, :, 1 : 1 + H, 1 : 1 + W] = x8
    tra = np.zeros((2, 128, PAD, PAD), ml_dtypes.float8_e4m3)
    tra[:, :, 1 : 1 + W, 1 : 1 + H] = x8.transpose(0, 1, 3, 2)
    full_n = np.zeros((2, 128, PADSTRIDE), ml_dtypes.float8_e4m3)
    full_n[:, :, :PADSZ] = nat.reshape(2, 128, PADSZ)
    full_t = np.zeros((2, 128, PADSTRIDE), ml_dtypes.float8_e4m3)
    full_t[:, :, :PADSZ] = tra.reshape(2, 128, PADSZ)
    return np.ascontiguousarray(full_n), np.ascontiguousarray(full_t)


def _run(inputs, trace=False, trace_kwargs=None):
    from concourse.bass_utils import run_bass_kernel_spmd

    nc = _get_program()
    x = np.ascontiguousarray(np.asarray(inputs["x"], np.float32))
    wpack = _pack_weights(inputs["Wq"], inputs["Wk"], inputs["Wv"])
    bq = np.asarray(inputs["bq"], np.float32)
    bk = np.asarray(inputs["bk"], np.float32)
    bv = np.asarray(inputs["bv"], np.float32)
    # col = occ*3 + cvslot, cvslot order (k, v, q)
    bpack = np.stack(
        [bk[:128], bv[:128], bq[:128], bk[128:], bv[128:], bq[128:]], axis=1
    )
    bpack = np.ascontiguousarray(bpack, dtype=np.float32)  # [128, 6]

    in_maps = []
    for b in range(N_CORES):
        x8n, x8t = _pack_x8(x[b])
        xbf = np.ascontiguousarray(
            x[b].astype(ml_dtypes.bfloat16).transpose(1, 0, 2)
        )
        in_maps.append(
            {"xbf": xbf, "x8n": x8n, "x8t": x8t, "wpack": wpack, "bpack": bpack}
        )
    last_err = None
    for attempt in range(3):
        try:
            res = run_bass_kernel_spmd(
                nc,
                in_maps,
                core_ids=list(range(N_CORES)),
                trace=trace,
                **(trace_kwargs or {}),
            )
            break
        except Exception as e:  # transient device/runtime hiccups
            last_err = e
            if attempt == 2:
                raise
            import time

            time.sleep(5.0)
    out = np.stack(
        [
            np.asarray(res.results[b]["y"], np.float32).transpose(1, 0, 2)
            for b in range(N_CORES)
        ],
        axis=0,
    )
    return out, res


def kernel(**inputs) -> np.ndarray:
    out, _ = _run(inputs, trace=False)
    return out


def kernel_traced(inputs):
    try:
        import axon_shim

        axon_shim.install()
    except Exception:
        pass
    out, res = _run(inputs, trace=True)
    return out, res


# revision 19
# speedup vs baseline: 1.0398x; 1.0148x over previous
"""Trainium2 Bass kernel for the AttentionConvBlock problem (fp8 conv edition).

Reference computation (per batch b of 8):
    q = relu(conv3x3(x, Wq) + bq); k = relu(conv3x3(x, Wk) + bk)
    v = conv3x3(x, Wv) + bv
    S = q @ k (per-channel [128,128] spatial matmul)
    P = softmax over flattened 16384 entries per (b, c)
    y = P @ v + x

Sharding: data-parallel over batch, one batch per NeuronCore (8 cores).

Per-core plan (vs the bf16 baseline, conv matmuls move to fp8 DoubleRow):
  - Host pre-quantizes x to fp8e4 (TRN E4M3, |x|max ~5 << 240) and uploads
    BOTH padded layouts (natural [h,w] and transposed [w,h]) with zero
    borders baked in: no on-device casts, no transpose pass, 4x less
    startup DMA than the fp32 staging path.
  - Weights are scaled by 2^13 (uniform(-1/48, 1/48) -> +-170, centered in
    e4m3 range) and packed per shift as [ic=128, icc=2, oc=128] fp8 tiles;
    the 2^-13 descale rides the PSUM-evacuation activation's scale.
  - Conv as 9-shift DoubleRow matmul: each shift contracts BOTH 128-channel
    input chunks at once (lhsT [128,2,128], rhs [128,2,L] 3-D APs, the PE
    virtualizes to 128x256). 9 matmuls/chunk instead of 18 at ~2 MACs/cell.
    Position chunks are FLAT runs of the padded image (3 rows x 130 = 390
    free dim <= 512 PSUM limit); the 2-column seam junk is simply never
    evacuated (strided ScalarE read of the valid 128 columns).
  - Chunks run in pairs sharing each shift's weight load (halves LDWEIGHTS
    pressure, which is 2x in DoubleRow); psum pool of 4 banks keeps the
    pair pipeline full while ScalarE drains evacuations.
  - q/k round-trip HBM in fp8 (attention S matmul runs fp8 at bf16 rate),
    v in bf16 (avoids mixed-dtype y matmul); P=exp(S) stays bf16.
  - Attention structure unchanged from the baseline: per 4-channel wave,
    4 S-matmuls into one PSUM bank, batched exp, DVE sums, all-ones-matmul
    global-sum broadcast, 4 y-matmuls, ScalarE 1/Z scale, DVE residual add.
    S and Y waves share one 3-bank PSUM tag ring. Block-0 attention is
    woven into block-1's conv stream; block-1's attention is the tail.
"""
import os
import sys

sys.path.insert(0, "/opt/trn_rl_repo")
os.environ.setdefault("MYCRO_LOCAL_CACHE", "1")

import numpy as np
import ml_dtypes

B, C, H, W = 8, 256, 128, 128
HW = H * W
N_CORES = 8
PAD = 130            # padded row/col length
PADSZ = PAD * PAD    # 16900 valid bytes per icc copy
PADSTRIDE = 16912    # icc stride, padded to %16 for the DoubleRow AP rule
WSCALE = 2.0 ** 13

# 43 position chunks per conv block: 42 x 3 rows + 1 x 2 rows
CHUNKS = [(r0, 3) for r0 in range(0, 126, 3)] + [(126, 2)]
GROUPS = [CHUNKS[i : i + 2] for i in range(0, len(CHUNKS), 2)]  # 21 pairs + single

_PROG = None


def _build_program():
    import concourse.bass as bass
    import concourse.tile as tile
    from concourse import bacc, mybir

    dt = mybir.dt
    AF = mybir.ActivationFunctionType
    DR = mybir.MatmulPerfMode.DoubleRow

    nc = bacc.Bacc("TRN2", target_bir_lowering=False, debug=False)

    # [H, C, W] so per-partition(h) runs are 8ch x 128 contiguous elements
    xbf_d = nc.dram_tensor("xbf", [H, C, W], dt.bfloat16, kind="ExternalInput").ap()
    x8n_d = nc.dram_tensor("x8n", [2, 128, PADSTRIDE], dt.float8e4, kind="ExternalInput").ap()
    x8t_d = nc.dram_tensor("x8t", [2, 128, PADSTRIDE], dt.float8e4, kind="ExternalInput").ap()
    w_d = nc.dram_tensor("wpack", [54, 128, 256], dt.float8e4, kind="ExternalInput").ap()
    b_d = nc.dram_tensor("bpack", [128, 6], dt.float32, kind="ExternalInput").ap()
    y_d = nc.dram_tensor("y", [H, C, W], dt.bfloat16, kind="ExternalOutput").ap()

    with tile.TileContext(nc) as tc:
        from contextlib import ExitStack

        with ExitStack() as ctx:
            const = ctx.enter_context(tc.tile_pool(name="const", bufs=1))
            xpad_p = ctx.enter_context(tc.tile_pool(name="xpad", bufs=1))
            evac = ctx.enter_context(tc.tile_pool(name="evac", bufs=4))
            qload = ctx.enter_context(tc.tile_pool(name="qload", bufs=6))
            kload = ctx.enter_context(tc.tile_pool(name="kload", bufs=6))
            vload = ctx.enter_context(tc.tile_pool(name="vload", bufs=6))
            att = ctx.enter_context(tc.tile_pool(name="att", bufs=2))
            stat = ctx.enter_context(tc.tile_pool(name="stat", bufs=3))
            resid = ctx.enter_context(tc.tile_pool(name="resid", bufs=6))
            outp = ctx.enter_context(tc.tile_pool(name="outp", bufs=3))
            psum_c = ctx.enter_context(tc.tile_pool(name="psc", bufs=4, space="PSUM"))
            psum_a = ctx.enter_context(tc.tile_pool(name="psa", bufs=3, space="PSUM"))
            psum_z = ctx.enter_context(tc.tile_pool(name="psz", bufs=1, space="PSUM"))
            dram = ctx.enter_context(tc.tile_pool(name="dram", bufs=1, space="DRAM"))

            # ---- constants ----
            w_sb = const.tile([128, 54, 256], dt.float8e4)
            b_sb = const.tile([128, 6], dt.float32)
            nc.scalar.dma_start(out=b_sb[:], in_=b_d)
            ones_f32 = const.tile([128, 128], dt.float32)
            nc.vector.memset(ones_f32[:], 1.0)

            # first shift's weights ahead of everything on the sync queue
            nc.sync.dma_start(
                out=w_sb[:, 0:1, :], in_=w_d[0:1].rearrange("t p f -> p t f")
            )

            # ---- padded fp8 x, natural and transposed (host pre-padded) ----
            # xa is the startup critical path (the k conv eats it at
            # ~65 GB/s): many mid-size slices round-robined over all three
            # queues so several DMA engines stream it in parallel. xt and
            # the late weights follow behind.
            xa = xpad_p.tile([128, 2, PADSTRIDE], dt.float8e4, tag="xa")
            xt = xpad_p.tile([128, 2, PADSTRIDE], dt.float8e4, tag="xt")
            queues = [nc.sync, nc.gpsimd, nc.scalar]
            NST = PADSTRIDE // 8  # 2114-byte slices
            nc.gpsimd.dma_start(
                out=w_sb[:, 1:9, :], in_=w_d[1:9].rearrange("t p f -> p t f")
            )
            for s in range(8):
                for icc in range(2):
                    queues[(2 * s + icc) % 3].dma_start(
                        out=xa[:, icc, s * NST : (s + 1) * NST],
                        in_=x8n_d[icc, :, s * NST : (s + 1) * NST],
                    )
                if s == 0:
                    nc.scalar.dma_start(
                        out=w_sb[:, 9:27, :], in_=w_d[9:27].rearrange("t p f -> p t f")
                    )
            for s in range(8):
                for icc in range(2):
                    queues[(2 * s + icc) % 3].dma_start(
                        out=xt[:, icc, s * NST : (s + 1) * NST],
                        in_=x8t_d[icc, :, s * NST : (s + 1) * NST],
                    )
            nc.scalar.dma_start(
                out=w_sb[:, 27:54, :], in_=w_d[27:54].rearrange("t p f -> p t f")
            )

            # ---- HBM round-trip buffers: position-major [j, c, i] ----
            qt_dram = dram.tile([128, C, 128], dt.float8e4, tag="qt")
            k_dram = dram.tile([128, C, 128], dt.float8e4, tag="kd")
            v_dram = dram.tile([128, C, 128], dt.float8e4, tag="vd")
            cv_dram = [k_dram, v_dram, qt_dram]  # cvslot order: k, v, q

            def conv_group(occ, cvslot, chunks):
                # one PSUM bank per chunk; shifts share each weight load
                src = xt if cvslot == 2 else xa
                ps = [
                    psum_c.tile([128, 3, PAD], dt.float32, tag="psc", name=f"psc{ci}")
                    for ci in range(len(chunks))
                ]
                for kk in range(9):
                    dy, dx = kk // 3, kk % 3
                    w3 = w_sb[:, occ * 27 + cvslot * 9 + kk, :].rearrange(
                        "p (two o) -> p two o", two=2
                    )
                    for ci, (r0, nr) in enumerate(chunks):
                        s0 = (r0 + dy) * PAD + dx
                        nc.tensor.matmul(
                            ps[ci][:, 0:nr, :],
                            lhsT=w3,
                            rhs=src[:, :, s0 : s0 + nr * PAD],
                            start=(kk == 0),
                            stop=(kk == 8),
                            perf_mode=DR,
                        )
                for ci, (r0, nr) in enumerate(chunks):
                    ev = evac.tile([128, 3, 128], dt.float8e4, tag="ev")
                    nc.scalar.activation(
                        out=ev[:, 0:nr, :],
                        in_=ps[ci][:, 0:nr, 0:128],
                        func=AF.Identity if cvslot == 1 else AF.Relu,
                        bias=b_sb[:, occ * 3 + cvslot : occ * 3 + cvslot + 1],
                        scale=1.0 / WSCALE,
                    )
                    nc.sync.dma_start(
                        out=cv_dram[cvslot][
                            r0 : r0 + nr, occ * 128 : (occ + 1) * 128, :
                        ].rearrange("j c i -> c j i"),
                        in_=ev[:, 0:nr, :],
                    )

            # ---- attention (baseline structure; q/k fp8, v bf16) ----
            def att_load(occ, g0):
                c0 = occ * 128 + g0
                qt8 = qload.tile([128, 8, 128], dt.float8e4, tag="qt8")
                nc.sync.dma_start(out=qt8[:], in_=qt_dram[:, c0 : c0 + 8, :])
                k8 = kload.tile([128, 8, 128], dt.float8e4, tag="k8")
                nc.sync.dma_start(out=k8[:], in_=k_dram[:, c0 : c0 + 8, :])
                v8 = vload.tile([128, 8, 128], dt.float8e4, tag="v8")
                nc.sync.dma_start(out=v8[:], in_=v_dram[:, c0 : c0 + 8, :])
                xr8 = resid.tile([128, 8, 128], dt.bfloat16, tag="xr8")
                nc.scalar.dma_start(out=xr8[:], in_=xbf_d[:, c0 : c0 + 8, :])
                return qt8, k8, v8, xr8

            def att_wave(occ, g0, tiles, w):
                qt8, k8, v8, xr8 = tiles
                o = 4 * w
                c0 = occ * 128 + g0 + o
                ps_s = psum_a.tile([128, 4, 128], dt.float32, tag="psa")
                for j in range(4):
                    nc.tensor.matmul(
                        ps_s[:, j, :],
                        lhsT=k8[:, o + j, :],
                        rhs=qt8[:, o + j, :],
                        start=True,
                        stop=True,
                    )
                p4 = att.tile([128, 4, 128], dt.bfloat16, tag="p4")
                nc.scalar.activation(out=p4[:], in_=ps_s[:], func=AF.Exp)
                cs4 = stat.tile([128, 4], dt.float32, tag="cs4")
                nc.vector.reduce_sum(cs4[:], p4[:], axis=mybir.AxisListType.X)
                ps_z = psum_z.tile([128, 4], dt.float32, tag="psz")
                nc.tensor.matmul(
                    ps_z[:], lhsT=ones_f32[:], rhs=cs4[:], start=True, stop=True
                )
                rec4 = stat.tile([128, 4], dt.float32, tag="rec4")
                nc.vector.reciprocal(rec4[:], ps_z[:])
                ps_y = psum_a.tile([128, 4, 128], dt.float32, tag="psa")
                for j in range(4):
                    nc.tensor.matmul(
                        ps_y[:, j, :],
                        lhsT=p4[:, j, :],
                        rhs=v8[:, o + j, :],
                        start=True,
                        stop=True,
                    )
                mul4 = outp.tile([128, 4, 128], dt.bfloat16, tag="mul4")
                nc.vector.tensor_mul(
                    mul4[:],
                    ps_y[:],
                    rec4[:, :, None].broadcast_to([128, 4, 128]),
                )
                out4 = outp.tile([128, 4, 128], dt.bfloat16, tag="out4")
                nc.gpsimd.tensor_add(out4[:], mul4[:], xr8[:, o : o + 4, :])
                nc.gpsimd.dma_start(out=y_d[:, c0 : c0 + 4, :], in_=out4[:])

            def att_steps(occ):
                for g0 in range(0, 128, 8):
                    tiles = []

                    def load(g0=g0, tiles=tiles):
                        tiles.append(att_load(occ, g0))

                    def wave0(g0=g0, tiles=tiles):
                        att_wave(occ, g0, tiles[0], 0)

                    def wave1(g0=g0, tiles=tiles):
                        att_wave(occ, g0, tiles[0], 1)

                    yield load
                    yield wave0
                    yield wave1

            # Phase A: block-0 convs (k, v, q order: xt arrives during k/v)
            for cvslot in range(3):
                for grp in GROUPS:
                    conv_group(0, cvslot, grp)
            # Phase B: block-1 convs with block-0 attention woven in
            steps = att_steps(0)
            for cvslot in range(3):
                for grp in GROUPS:
                    conv_group(1, cvslot, grp)
                    step = next(steps, None)
                    if step is not None:
                        step()
            for step in steps:
                step()
            # Phase C: block-1 attention tail
            for step in att_steps(1):
                step()

    nc.compile()
    return nc


def _get_program():
    global _PROG
    if _PROG is None:
        _PROG = _build_program()
    return _PROG


def _pack_weights(Wq, Wk, Wv):
    # w_d[t, ic, icc*128 + oc], t = occ*27 + cvslot*9 + kk (cvslot: k,v,q).
    # The q conv runs on the TRANSPOSED image with the same (dy,dx) shift
    # arithmetic, so its taps must be packed transposed.
    out = np.zeros((54, 128, 256), np.float32)
    for cvslot, Wcv in ((0, Wk), (1, Wv), (2, np.asarray(Wq).transpose(0, 1, 3, 2))):
        a = np.asarray(Wcv, np.float32) * WSCALE  # [ocg, icg, dy, dx]
        a = a.transpose(2, 3, 1, 0).reshape(9, 2, 128, 2, 128)  # kk,icc,ic,occ,oc
        a = a.transpose(3, 0, 2, 1, 4).reshape(2, 9, 128, 256)  # occ,kk,ic,(icc oc)
        for occ in range(2):
            base = occ * 27 + cvslot * 9
            out[base : base + 9] = a[occ]
    return np.clip(out, -240, 240).astype(ml_dtypes.float8_e4m3)


def _pack_x8(xb):
    # xb [256, 128, 128] fp32 -> padded fp8 natural + transposed layouts
    x8 = np.asarray(xb, np.float32).astype(ml_dtypes.float8_e4m3)
    x8 = x8.reshape(2, 128, H, W)
    nat = np.zeros((2, 128, PAD, PAD), ml_dtypes.float8_e4m3)
    nat[:, :, 1 : 1 + H, 1 : 1 + W] = x8
    tra = np.zeros((2, 128, PAD, PAD), ml_dtypes.float8_e4m3)
    tra[:, :, 1 : 1 + W, 1 : 1 + H] = x8.transpose(0, 1, 3, 2)
    full_n = np.zeros((2, 128, PADSTRIDE), ml_dtypes.float8_e4m3)
    full_n[:, :, :PADSZ] = nat.reshape(2, 128, PADSZ)
    full_t = np.zeros((2, 128, PADSTRIDE), ml_dtypes.float8_e4m3)
    full_t[:, :, :PADSZ] = tra.reshape(2, 128, PADSZ)
    return np.ascontiguousarray(full_n), np.ascontiguousarray(full_t)


def _run(inputs, trace=False, trace_kwargs=None):
    from concourse.bass_utils import run_bass_kernel_spmd

    nc = _get_program()
    x = np.ascontiguousarray(np.asarray(inputs["x"], np.float32))
    wpack = _pack_weights(inputs["Wq"], inputs["Wk"], inputs["Wv"])
    bq = np.asarray(inputs["bq"], np.float32)
    bk = np.asarray(inputs["bk"], np.float32)
    bv = np.asarray(inputs["bv"], np.float32)
    # col = occ*3 + cvslot, cvslot order (k, v, q)
    bpack = np.stack(
        [bk[:128], bv[:128], bq[:128], bk[128:], bv[128:], bq[128:]], axis=1
    )
    bpack = np.ascontiguousarray(bpack, dtype=np.float32)  # [128, 6]

    in_maps = []
    for b in range(N_CORES):
        x8n, x8t = _pack_x8(x[b])
        xbf = np.ascontiguousarray(
            x[b].astype(ml_dtypes.bfloat16).transpose(1, 0, 2)
        )
        in_maps.append(
            {"xbf": xbf, "x8n": x8n, "x8t": x8t, "wpack": wpack, "bpack": bpack}
        )
    last_err = None
    for attempt in range(3):
        try:
            res = run_bass_kernel_spmd(
                nc,
                in_maps,
                core_ids=list(range(N_CORES)),
                trace=trace,
                **(trace_kwargs or {}),
            )
            break
        except Exception as e:  # transient device/runtime hiccups
            last_err = e
            if attempt == 2:
                raise
            import time

            time.sleep(5.0)
    out = np.stack(
        [
            np.asarray(res.results[b]["y"], np.float32).transpose(1, 0, 2)
            for b in range(N_CORES)
        ],
        axis=0,
    )
    return out, res


def kernel(**inputs) -> np.ndarray:
    out, _ = _run(inputs, trace=False)
    return out


def kernel_traced(inputs):
    try:
        import axon_shim

        axon_shim.install()
    except Exception:
        pass
    out, res = _run(inputs, trace=True)
    return out, res


# revision 21
# speedup vs baseline: 1.0686x; 1.0277x over previous
"""Trainium2 Bass kernel for the AttentionConvBlock problem (fp8 conv edition).

Reference computation (per batch b of 8):
    q = relu(conv3x3(x, Wq) + bq); k = relu(conv3x3(x, Wk) + bk)
    v = conv3x3(x, Wv) + bv
    S = q @ k (per-channel [128,128] spatial matmul)
    P = softmax over flattened 16384 entries per (b, c)
    y = P @ v + x

Sharding: data-parallel over batch, one batch per NeuronCore (8 cores).

Per-core plan (vs the bf16 baseline, conv matmuls move to fp8 DoubleRow):
  - Host pre-quantizes x to fp8e4 (TRN E4M3, |x|max ~5 << 240) and uploads
    BOTH padded layouts (natural [h,w] and transposed [w,h]) with zero
    borders baked in: no on-device casts, no transpose pass, 4x less
    startup DMA than the fp32 staging path.
  - Weights are scaled by 2^13 (uniform(-1/48, 1/48) -> +-170, centered in
    e4m3 range) and packed per shift as [ic=128, icc=2, oc=128] fp8 tiles;
    the 2^-13 descale rides the PSUM-evacuation activation's scale.
  - Conv as 9-shift DoubleRow matmul: each shift contracts BOTH 128-channel
    input chunks at once (lhsT [128,2,128], rhs [128,2,L] 3-D APs, the PE
    virtualizes to 128x256). 9 matmuls/chunk instead of 18 at ~2 MACs/cell.
    Position chunks are FLAT runs of the padded image (3 rows x 130 = 390
    free dim <= 512 PSUM limit); the 2-column seam junk is simply never
    evacuated (strided ScalarE read of the valid 128 columns).
  - Chunks run in pairs sharing each shift's weight load (halves LDWEIGHTS
    pressure, which is 2x in DoubleRow); psum pool of 4 banks keeps the
    pair pipeline full while ScalarE drains evacuations.
  - q/k round-trip HBM in fp8 (attention S matmul runs fp8 at bf16 rate),
    v in bf16 (avoids mixed-dtype y matmul); P=exp(S) stays bf16.
  - Attention structure unchanged from the baseline: per 4-channel wave,
    4 S-matmuls into one PSUM bank, batched exp, DVE sums, all-ones-matmul
    global-sum broadcast, 4 y-matmuls, ScalarE 1/Z scale, DVE residual add.
    S and Y waves share one 3-bank PSUM tag ring. Block-0 attention is
    woven into block-1's conv stream; block-1's attention is the tail.
"""
import os
import sys

sys.path.insert(0, "/opt/trn_rl_repo")
os.environ.setdefault("MYCRO_LOCAL_CACHE", "1")

import numpy as np
import ml_dtypes

B, C, H, W = 8, 256, 128, 128
HW = H * W
N_CORES = 8
PAD = 130            # padded row/col length
PADSZ = PAD * PAD    # 16900 valid bytes per icc copy
PADSTRIDE = 16912    # icc stride, padded to %16 for the DoubleRow AP rule
WSCALE = 2.0 ** 13

# 43 position chunks per conv block: 42 x 3 rows + 1 x 2 rows
CHUNKS = [(r0, 3) for r0 in range(0, 126, 3)] + [(126, 2)]
GROUPS = [CHUNKS[i : i + 2] for i in range(0, len(CHUNKS), 2)]  # 21 pairs + single

_PROG = None


def _build_program():
    import concourse.bass as bass
    import concourse.tile as tile
    from concourse import bacc, mybir

    dt = mybir.dt
    AF = mybir.ActivationFunctionType
    DR = mybir.MatmulPerfMode.DoubleRow

    nc = bacc.Bacc("TRN2", target_bir_lowering=False, debug=False)

    # [H, C, W] so per-partition(h) runs are 8ch x 128 contiguous elements
    xbf_d = nc.dram_tensor("xbf", [H, C, W], dt.bfloat16, kind="ExternalInput").ap()
    x8n_d = nc.dram_tensor("x8n", [2, 128, PADSTRIDE], dt.float8e4, kind="ExternalInput").ap()
    x8t_d = nc.dram_tensor("x8t", [2, 128, PADSTRIDE], dt.float8e4, kind="ExternalInput").ap()
    w_d = nc.dram_tensor("wpack", [54, 128, 256], dt.float8e4, kind="ExternalInput").ap()
    b_d = nc.dram_tensor("bpack", [128, 6], dt.float32, kind="ExternalInput").ap()
    y_d = nc.dram_tensor("y", [H, C, W], dt.bfloat16, kind="ExternalOutput").ap()

    with tile.TileContext(nc) as tc:
        from contextlib import ExitStack

        with ExitStack() as ctx:
            const = ctx.enter_context(tc.tile_pool(name="const", bufs=1))
            xpad_p = ctx.enter_context(tc.tile_pool(name="xpad", bufs=1))
            evac = ctx.enter_context(tc.tile_pool(name="evac", bufs=4))
            qload = ctx.enter_context(tc.tile_pool(name="qload", bufs=6))
            kload = ctx.enter_context(tc.tile_pool(name="kload", bufs=6))
            vload = ctx.enter_context(tc.tile_pool(name="vload", bufs=6))
            att = ctx.enter_context(tc.tile_pool(name="att", bufs=2))
            stat = ctx.enter_context(tc.tile_pool(name="stat", bufs=3))
            resid = ctx.enter_context(tc.tile_pool(name="resid", bufs=6))
            outp = ctx.enter_context(tc.tile_pool(name="outp", bufs=3))
            psum_c = ctx.enter_context(tc.tile_pool(name="psc", bufs=4, space="PSUM"))
            psum_a = ctx.enter_context(tc.tile_pool(name="psa", bufs=3, space="PSUM"))
            psum_z = ctx.enter_context(tc.tile_pool(name="psz", bufs=1, space="PSUM"))
            dram = ctx.enter_context(tc.tile_pool(name="dram", bufs=1, space="DRAM"))

            # ---- constants ----
            w_sb = const.tile([128, 54, 256], dt.float8e4)
            b_sb = const.tile([128, 6], dt.float32)
            nc.scalar.dma_start(out=b_sb[:], in_=b_d)
            ones_f32 = const.tile([128, 128], dt.float32)
            nc.vector.memset(ones_f32[:], 1.0)

            # first shift's weights ahead of everything on the sync queue
            nc.sync.dma_start(
                out=w_sb[:, 0:1, :], in_=w_d[0:1].rearrange("t p f -> p t f")
            )

            # ---- padded fp8 x, natural and transposed (host pre-padded) ----
            # xa is the startup critical path (the k conv eats it at
            # ~65 GB/s): many mid-size slices round-robined over all three
            # queues so several DMA engines stream it in parallel. xt and
            # the late weights follow behind.
            xa = xpad_p.tile([128, 2, PADSTRIDE], dt.float8e4, tag="xa")
            xt = xpad_p.tile([128, 2, PADSTRIDE], dt.float8e4, tag="xt")
            queues = [nc.sync, nc.gpsimd, nc.scalar]
            NST = PADSTRIDE // 8  # 2114-byte slices
            for s in range(8):
                for icc in range(2):
                    queues[(2 * s + icc) % 3].dma_start(
                        out=xa[:, icc, s * NST : (s + 1) * NST],
                        in_=x8n_d[icc, :, s * NST : (s + 1) * NST],
                    )
                if s == 0:
                    # k-conv shifts 1..8: needed ~1.5us after the first MM
                    nc.gpsimd.dma_start(
                        out=w_sb[:, 1:9, :], in_=w_d[1:9].rearrange("t p f -> p t f")
                    )
            # v-conv weights: not needed until ~1/3 into phase A — keep them
            # strictly behind every xa slice so they never delay the k conv
            nc.scalar.dma_start(
                out=w_sb[:, 9:27, :], in_=w_d[9:27].rearrange("t p f -> p t f")
            )
            for s in range(8):
                for icc in range(2):
                    queues[(2 * s + icc) % 3].dma_start(
                        out=xt[:, icc, s * NST : (s + 1) * NST],
                        in_=x8t_d[icc, :, s * NST : (s + 1) * NST],
                    )
            nc.scalar.dma_start(
                out=w_sb[:, 27:54, :], in_=w_d[27:54].rearrange("t p f -> p t f")
            )

            # ---- HBM round-trip buffers: position-major [j, c, i] ----
            qt_dram = dram.tile([128, C, 128], dt.float8e4, tag="qt")
            k_dram = dram.tile([128, C, 128], dt.float8e4, tag="kd")
            v_dram = dram.tile([128, C, 128], dt.float8e4, tag="vd")
            cv_dram = [k_dram, v_dram, qt_dram]  # cvslot order: k, v, q

            def conv_group(occ, cvslot, chunks):
                # one PSUM bank per chunk; shifts share each weight load
                src = xt if cvslot == 2 else xa
                ps = [
                    psum_c.tile([128, 3, PAD], dt.float32, tag="psc", name=f"psc{ci}")
                    for ci in range(len(chunks))
                ]
                for kk in range(9):
                    dy, dx = kk // 3, kk % 3
                    w3 = w_sb[:, occ * 27 + cvslot * 9 + kk, :].rearrange(
                        "p (two o) -> p two o", two=2
                    )
                    for ci, (r0, nr) in enumerate(chunks):
                        s0 = (r0 + dy) * PAD + dx
                        nc.tensor.matmul(
                            ps[ci][:, 0:nr, :],
                            lhsT=w3,
                            rhs=src[:, :, s0 : s0 + nr * PAD],
                            start=(kk == 0),
                            stop=(kk == 8),
                            perf_mode=DR,
                        )
                for ci, (r0, nr) in enumerate(chunks):
                    ev = evac.tile([128, 3, 128], dt.float8e4, tag="ev")
                    nc.scalar.activation(
                        out=ev[:, 0:nr, :],
                        in_=ps[ci][:, 0:nr, 0:128],
                        func=AF.Identity if cvslot == 1 else AF.Relu,
                        bias=b_sb[:, occ * 3 + cvslot : occ * 3 + cvslot + 1],
                        scale=1.0 / WSCALE,
                    )
                    nc.sync.dma_start(
                        out=cv_dram[cvslot][
                            r0 : r0 + nr, occ * 128 : (occ + 1) * 128, :
                        ].rearrange("j c i -> c j i"),
                        in_=ev[:, 0:nr, :],
                    )

            # ---- attention (baseline structure; q/k fp8, v bf16) ----
            def att_load(occ, g0):
                c0 = occ * 128 + g0
                qt8 = qload.tile([128, 8, 128], dt.float8e4, tag="qt8")
                nc.sync.dma_start(out=qt8[:], in_=qt_dram[:, c0 : c0 + 8, :])
                k8 = kload.tile([128, 8, 128], dt.float8e4, tag="k8")
                nc.sync.dma_start(out=k8[:], in_=k_dram[:, c0 : c0 + 8, :])
                v8 = vload.tile([128, 8, 128], dt.float8e4, tag="v8")
                nc.sync.dma_start(out=v8[:], in_=v_dram[:, c0 : c0 + 8, :])
                xr8 = resid.tile([128, 8, 128], dt.bfloat16, tag="xr8")
                nc.scalar.dma_start(out=xr8[:], in_=xbf_d[:, c0 : c0 + 8, :])
                return qt8, k8, v8, xr8

            def att_wave(occ, g0, tiles, w):
                qt8, k8, v8, xr8 = tiles
                o = 4 * w
                c0 = occ * 128 + g0 + o
                ps_s = psum_a.tile([128, 4, 128], dt.float32, tag="psa")
                for j in range(4):
                    nc.tensor.matmul(
                        ps_s[:, j, :],
                        lhsT=k8[:, o + j, :],
                        rhs=qt8[:, o + j, :],
                        start=True,
                        stop=True,
                    )
                p4 = att.tile([128, 4, 128], dt.bfloat16, tag="p4")
                nc.scalar.activation(out=p4[:], in_=ps_s[:], func=AF.Exp)
                cs4 = stat.tile([128, 4], dt.float32, tag="cs4")
                nc.vector.reduce_sum(cs4[:], p4[:], axis=mybir.AxisListType.X)
                ps_z = psum_z.tile([128, 4], dt.float32, tag="psz")
                nc.tensor.matmul(
                    ps_z[:], lhsT=ones_f32[:], rhs=cs4[:], start=True, stop=True
                )
                rec4 = stat.tile([128, 4], dt.float32, tag="rec4")
                nc.vector.reciprocal(rec4[:], ps_z[:])
                ps_y = psum_a.tile([128, 4, 128], dt.float32, tag="psa")
                for j in range(4):
                    nc.tensor.matmul(
                        ps_y[:, j, :],
                        lhsT=p4[:, j, :],
                        rhs=v8[:, o + j, :],
                        start=True,
                        stop=True,
                    )
                mul4 = outp.tile([128, 4, 128], dt.bfloat16, tag="mul4")
                nc.vector.tensor_mul(
                    mul4[:],
                    ps_y[:],
                    rec4[:, :, None].broadcast_to([128, 4, 128]),
                )
                out4 = outp.tile([128, 4, 128], dt.bfloat16, tag="out4")
                nc.gpsimd.tensor_add(out4[:], mul4[:], xr8[:, o : o + 4, :])
                nc.scalar.dma_start(out=y_d[:, c0 : c0 + 4, :], in_=out4[:])

            def att_steps(occ):
                for g0 in range(0, 128, 8):
                    tiles = []

                    def load(g0=g0, tiles=tiles):
                        tiles.append(att_load(occ, g0))

                    def wave0(g0=g0, tiles=tiles):
                        att_wave(occ, g0, tiles[0], 0)

                    def wave1(g0=g0, tiles=tiles):
                        att_wave(occ, g0, tiles[0], 1)

                    yield load
                    yield wave0
                    yield wave1

            # Phase A: block-0 convs (k, v, q order: xt arrives during k/v)
            for cvslot in range(3):
                for grp in GROUPS:
                    conv_group(0, cvslot, grp)
            # Phase B: block-1 convs with block-0 attention woven in
            steps = att_steps(0)
            for cvslot in range(3):
                for grp in GROUPS:
                    conv_group(1, cvslot, grp)
                    step = next(steps, None)
                    if step is not None:
                        step()
            for step in steps:
                step()
            # Phase C: block-1 attention tail
            for step in att_steps(1):
                step()

    nc.compile()
    return nc


def _get_program():
    global _PROG
    if _PROG is None:
        _PROG = _build_program()
    return _PROG


def _pack_weights(Wq, Wk, Wv):
    # w_d[t, ic, icc*128 + oc], t = occ*27 + cvslot*9 + kk (cvslot: k,v,q).
    # The q conv runs on the TRANSPOSED image with the same (dy,dx) shift
    # arithmetic, so its taps must be packed transposed.
    out = np.zeros((54, 128, 256), np.float32)
    for cvslot, Wcv in ((0, Wk), (1, Wv), (2, np.asarray(Wq).transpose(0, 1, 3, 2))):
        a = np.asarray(Wcv, np.float32) * WSCALE  # [ocg, icg, dy, dx]
        a = a.transpose(2, 3, 1, 0).reshape(9, 2, 128, 2, 128)  # kk,icc,ic,occ,oc
        a = a.transpose(3, 0, 2, 1, 4).reshape(2, 9, 128, 256)  # occ,kk,ic,(icc oc)
        for occ in range(2):
            base = occ * 27 + cvslot * 9
            out[base : base + 9] = a[occ]
    return np.clip(out, -240, 240).astype(ml_dtypes.float8_e4m3)


def _pack_x8(xb):
    # xb [256, 128, 128] fp32 -> padded fp8 natural + transposed layouts
    x8 = np.asarray(xb, np.float32).astype(ml_dtypes.float8_e4m3)
    x8 = x8.reshape(2, 128, H, W)
    nat = np.zeros((2, 128, PAD, PAD), ml_dtypes.float8_e4m3)
    nat[:, :, 1 : 1 + H, 1 : 1 + W] = x8
    tra = np.zeros((2, 128, PAD, PAD), ml_dtypes.float8_e4m3)
    tra[:, :, 1 : 1 + W, 1 : 1 + H] = x8.transpose(0, 1, 3, 2)
    full_n = np.zeros((2, 128, PADSTRIDE), ml_dtypes.float8_e4m3)
    full_n[:, :, :PADSZ] = nat.reshape(2, 128, PADSZ)
    full_t = np.zeros((2, 128, PADSTRIDE), ml_dtypes.float8_e4m3)
    full_t[:, :, :PADSZ] = tra.reshape(2, 128, PADSZ)
    return np.ascontiguousarray(full_n), np.ascontiguousarray(full_t)


def _run(inputs, trace=False, trace_kwargs=None):
    from concourse.bass_utils import run_bass_kernel_spmd

    nc = _get_program()
    x = np.ascontiguousarray(np.asarray(inputs["x"], np.float32))
    wpack = _pack_weights(inputs["Wq"], inputs["Wk"], inputs["Wv"])
    bq = np.asarray(inputs["bq"], np.float32)
    bk = np.asarray(inputs["bk"], np.float32)
    bv = np.asarray(inputs["bv"], np.float32)
    # col = occ*3 + cvslot, cvslot order (k, v, q)
    bpack = np.stack(
        [bk[:128], bv[:128], bq[:128], bk[128:], bv[128:], bq[128:]], axis=1
    )
    bpack = np.ascontiguousarray(bpack, dtype=np.float32)  # [128, 6]

    in_maps = []
    for b in range(N_CORES):
        x8n, x8t = _pack_x8(x[b])
        xbf = np.ascontiguousarray(
            x[b].astype(ml_dtypes.bfloat16).transpose(1, 0, 2)
        )
        in_maps.append(
            {"xbf": xbf, "x8n": x8n, "x8t": x8t, "wpack": wpack, "bpack": bpack}
        )
    last_err = None
    for attempt in range(3):
        try:
            res = run_bass_kernel_spmd(
                nc,
                in_maps,
                core_ids=list(range(N_CORES)),
                trace=trace,
                **(trace_kwargs or {}),
            )
            break
        except Exception as e:  # transient device/runtime hiccups
            last_err = e
            if attempt == 2:
                raise
            import time

            time.sleep(5.0)
    out = np.stack(
        [
            np.asarray(res.results[b]["y"], np.float32).transpose(1, 0, 2)
            for b in range(N_CORES)
        ],
        axis=0,
    )
    return out, res


def kernel(**inputs) -> np.ndarray:
    out, _ = _run(inputs, trace=False)
    return out


def kernel_traced(inputs):
    try:
        import axon_shim

        axon_shim.install()
    except Exception:
        pass
    out, res = _run(inputs, trace=True)
    return out, res
